# revision 12
# baseline (speedup 1.0000x reference)
"""Trainium2 Bass kernel for nn_MeSH_GCN (CNN-attention + 2-layer label GCN).

Self-contained: hardcodes shapes, shards across 8 NeuronCores internally.
Strategy: nodes (N=28000) degree-sorted and dealt round-robin to the 8 cores
(3500 real + 84 pad nodes each). Each core computes the full CNN/attention
path and both GCN layers for its own nodes; an on-chip AllGather shares the
first-layer GCN output between cores. The host only prepares index/layout
arrays and reassembles the output.
"""
import numpy as np

import concourse.bacc as bacc
import concourse.bass as bass
import concourse.mybir as mybir
import concourse.tile as tile
from concourse.masks import make_identity

B, L, V, D = 4, 512, 50000, 200
N, E = 28000, 448000
NK, HG = 128, 512
KSZ = (3, 4, 5)
NCORES = 8
NT = 28                      # node tiles of 128 per core
NPC = NT * 128               # 3584 padded nodes per core
NREAL = N // NCORES          # 3500 real nodes per core
PADF = 256                   # padded feature row (256 f32 = 1KB)
NFP = 28016                  # rows in padded features table (>= N+1, %16)
ZROW1 = N                    # zero row in features_pad
ZROW2 = 3583                 # zero row in hfull (pad region of core 0)
JMAX = 8                     # gather-op width (columns of 128 edges)

F32 = mybir.dt.float32
F32R = mybir.dt.float32r
I16 = mybir.dt.int16
I32 = mybir.dt.int32
AF = mybir.ActivationFunctionType
ALU = mybir.AluOpType
AX = mybir.AxisListType


# ---------------------------------------------------------------- host prep
def _edge_layout(src, dst):
    """Degree-sorted round-robin node assignment + per-core padded edge
    matrices grouped into fixed-width gather ops (identical structure on
    every core so one SPMD program works)."""
    deg = np.bincount(dst, minlength=N)
    order = np.argsort(-deg, kind="stable").astype(np.int64)
    # gpos[v]: row of node v in the allgathered table
    rank_of = np.empty(N, dtype=np.int64)
    rank_of[order] = np.arange(N)
    gpos = (rank_of % NCORES) * NPC + rank_of // NCORES

    eidx = np.argsort(dst, kind="stable")
    sdst = dst[eidx]
    starts = np.searchsorted(sdst, np.arange(N + 1))
    esrc = src[eidx]

    # K per tile: max degree among global ranks [t*1024, (t+1)*1024)
    Ks = []
    for t in range(NT):
        r0, r1 = t * 128 * NCORES, min((t + 1) * 128 * NCORES, N)
        Ks.append(max(1, int(deg[order[r0:r1]].max()) if r1 > r0 else 1))
    assert all(k <= 8 * JMAX for k in Ks)

    # greedy op grouping (close current op before a tile wider than JMAX)
    segs = [[] for _ in range(NT)]  # per tile: (op, col_off, ncols)
    ops_cols = []
    cur = 0
    for t in range(NT):
        rem = Ks[t]
        if rem > JMAX and cur > 0:
            ops_cols.append(cur)
            cur = 0
        while rem > 0:
            take = min(rem, JMAX - cur)
            segs[t].append((len(ops_cols), cur, take))
            cur += take
            rem -= take
            if cur == JMAX:
                ops_cols.append(JMAX)
                cur = 0
    if cur > 0:
        ops_cols.append(cur)

    # per-core padded edge matrices -> wrapped int16 index arrays per phase
    TC = sum(Ks)
    W1 = np.zeros((NCORES, 128, 8 * TC), np.int16)
    W2 = np.zeros((NCORES, 128, 8 * TC), np.int16)
    col2tile = []
    for t in range(NT):
        col2tile += [t] * Ks[t]
    for c in range(NCORES):
        M1 = np.full((TC, 128), ZROW1, np.int16)
        M2 = np.full((TC, 128), ZROW2, np.int16)
        co = 0
        for t in range(NT):
            for p in range(128):
                i = t * 128 + p
                if i < NREAL:
                    v = order[i * NCORES + c]
                    ee = esrc[starts[v]:starts[v + 1]]
                    M1[co:co + len(ee), p] = ee
                    M2[co:co + len(ee), p] = gpos[ee]
            co += Ks[t]
        for Msrc, Wdst in ((M1, W1), (M2, W2)):
            flat = Msrc.reshape(-1)  # col-major: [TC,128] -> idx j*128+p
            Wdst[c] = np.tile(flat.reshape(-1, 16).T, (8, 1))
    return order, Ks, segs, ops_cols, W1, W2


def _host_prep(inputs):
    src = np.asarray(inputs["src"]); dst = np.asarray(inputs["dst"])
    feats = np.asarray(inputs["features"], np.float32)
    feat = np.asarray(inputs["feat"], np.float32)
    emb = np.asarray(inputs["emb"], np.float32)
    iseq = np.asarray(inputs["input_seq"], np.int32).reshape(-1, 1)

    order, Ks, segs, ops_cols, W1, W2 = _edge_layout(src, dst)

    fpad = np.zeros((NFP, PADF), np.float32)
    fpad[:N, :D] = feats

    consts = np.zeros((128, 209), np.float32)
    consts[:, 0] = inputs["bc3"]; consts[:, 1] = inputs["bc4"]; consts[:, 2] = inputs["bc5"]
    bt = np.asarray(inputs["bt"], np.float32)
    consts[:, 3] = bt[:128]; consts[:72, 4] = bt[128:]
    consts[:, 5:9] = np.asarray(inputs["bg1"], np.float32).reshape(4, 128).T
    consts[:, 9:209] = np.tile(np.asarray(inputs["bg2"], np.float32), (128, 1))

    wc = {k: np.ascontiguousarray(
            np.asarray(inputs[f"Wc{k}"], np.float32)[:, 0].transpose(1, 2, 0))
          for k in KSZ}          # [k, D, NK] -> stored [k, 200, 128]
    wfT = np.ascontiguousarray(np.asarray(inputs["Wf"], np.float32).T)      # [384,400]
    wtT = np.ascontiguousarray(np.asarray(inputs["Wt"], np.float32).T)      # [128,200]
    wg1T = np.ascontiguousarray(np.asarray(inputs["Wg1"], np.float32).T)    # [200,512]
    wg2T = np.ascontiguousarray(np.asarray(inputs["Wg2"], np.float32).T)    # [512,200]
    bf_row = np.asarray(inputs["bf"], np.float32).reshape(1, 400)

    in_maps = []
    for c in range(NCORES):
        nodes = order[c::NCORES]                       # 3500 real nodes
        fT = np.zeros((200, NPC), np.float32)
        fT[:, :NREAL] = feat[nodes].T
        fs = np.zeros((NPC, 200), np.float32)
        fs[:NREAL] = feat[nodes]
        in_maps.append({
            "fpad": fpad, "emb": emb, "iseq": iseq,
            "W1": W1[c], "W2": W2[c],
            "featT": fT, "feat_s": fs,
            "consts": consts, "bf_row": bf_row,
            "wc3": wc[3], "wc4": wc[4], "wc5": wc[5],
            "wfT": wfT, "wtT": wtT, "wg1T": wg1T, "wg2T": wg2T,
            "ones_c": np.ones((128, 1), np.float32),
            "ones_rw": np.ones((1, 128), np.float32),
        })
    meta = dict(Ks=Ks, segs=segs, ops_cols=ops_cols, order=order)
    return in_maps, meta


# ---------------------------------------------------------------- device
def build_program(meta, debug=False, parts="all"):
    Ks, segs, ops_cols = meta["Ks"], meta["segs"], meta["ops_cols"]
    nc = bacc.Bacc("TRN2", target_bir_lowering=False, debug=False,
                   num_devices=NCORES)

    fpad = nc.dram_tensor("fpad", [NFP, PADF], F32, kind="ExternalInput").ap()
    emb = nc.dram_tensor("emb", [V, D], F32, kind="ExternalInput").ap()
    iseq = nc.dram_tensor("iseq", [B * L, 1], I32, kind="ExternalInput").ap()
    W1 = nc.dram_tensor("W1", list(meta_W_shape(Ks)), I16, kind="ExternalInput").ap()
    W2 = nc.dram_tensor("W2", list(meta_W_shape(Ks)), I16, kind="ExternalInput").ap()
    featT = nc.dram_tensor("featT", [200, NPC], F32R, kind="ExternalInput").ap()
    feat_s = nc.dram_tensor("feat_s", [NPC, 200], F32, kind="ExternalInput").ap()
    consts = nc.dram_tensor("consts", [128, 209], F32, kind="ExternalInput").ap()
    bf_row = nc.dram_tensor("bf_row", [1, 400], F32R, kind="ExternalInput").ap()
    wcd = {k: nc.dram_tensor(f"wc{k}", [k, 200, 128], F32, kind="ExternalInput").ap()
           for k in KSZ}
    wfTd = nc.dram_tensor("wfT", [384, 400], F32R, kind="ExternalInput").ap()
    wtTd = nc.dram_tensor("wtT", [128, 200], F32, kind="ExternalInput").ap()
    wg1Td = nc.dram_tensor("wg1T", [200, 512], F32R, kind="ExternalInput").ap()
    wg2Td = nc.dram_tensor("wg2T", [512, 200], F32, kind="ExternalInput").ap()
    ones_c = nc.dram_tensor("ones_c", [128, 1], F32R, kind="ExternalInput").ap()
    ones_rw = nc.dram_tensor("ones_rw", [1, 128], F32R, kind="ExternalInput").ap()
    out = nc.dram_tensor("out", [NPC, B], F32, kind="ExternalOutput").ap()
    dbg = {}
    if debug:
        dbg["h2"] = nc.dram_tensor("dbg_h2", [128, NT * 200], F32, kind="ExternalOutput").ap()
        dbg["hb"] = nc.dram_tensor("dbg_hb", [NPC, PADF], F32, kind="ExternalOutput").ap()
        dbg["xc"] = nc.dram_tensor("dbg_xc", [128, 3 * 512], F32, kind="ExternalOutput").ap()
        dbg["xf"] = nc.dram_tensor("dbg_xf", [128, 400], F32, kind="ExternalOutput").ap()

    with tile.TileContext(nc) as tc:
        _emit(nc, tc, locals(), meta, debug, dbg, parts)
    nc.compile()
    return nc


def meta_W_shape(Ks):
    return (128, 8 * sum(Ks))


def _emit(nc, tc, T, meta, debug, dbg, parts="all"):
    Ks, segs, ops_cols = meta["Ks"], meta["segs"], meta["ops_cols"]
    fpad, emb, iseq, W1, W2 = T["fpad"], T["emb"], T["iseq"], T["W1"], T["W2"]
    featT, feat_s, consts_d, bf_row = T["featT"], T["feat_s"], T["consts"], T["bf_row"]
    wcd, wfTd, wtTd, wg1Td, wg2Td, out = (T["wcd"], T["wfTd"], T["wtTd"],
                                          T["wg1Td"], T["wg2Td"], T["out"])

    with tc.tile_pool(name="pconst", bufs=1) as pc, \
         tc.tile_pool(name="pdram", bufs=1, space="DRAM") as pd, \
         tc.tile_pool(name="pgat", bufs=2) as pg, \
         tc.tile_pool(name="pidx", bufs=2) as pi, \
         tc.tile_pool(name="pwork", bufs=2) as pw, \
         tc.tile_pool(name="pbig", bufs=1) as pb, \
         tc.tile_pool(name="psA", bufs=3, space="PSUM") as psA, \
         tc.tile_pool(name="psB", bufs=2, space="PSUM") as psB, \
         tc.tile_pool(name="psG", bufs=2, space="PSUM") as psG:

        # ---------------- constants
        cst = pc.tile([128, 209], F32, name="cst")
        nc.sync.dma_start(out=cst[:], in_=consts_d[:])
        ident = pc.tile([128, 128], F32, name="ident")
        make_identity(nc, ident[:])
        ones_r = pc.tile([128, 1], F32R, name="ones_r")
        nc.sync.dma_start(out=ones_r[:], in_=T["ones_c"][:])
        ones_row = pc.tile([1, 128], F32R, name="ones_row")
        nc.sync.dma_start(out=ones_row[:], in_=T["ones_rw"][:])
        zeros_t = pc.tile([128, PADF], F32, name="zeros_t")
        nc.vector.memset(zeros_t[:], 0.0)
        bfr = pc.tile([1, 400], F32R, name="bfr")
        nc.sync.dma_start(out=bfr[:], in_=bf_row[:])
        wfT = pc.tile([128, 3, 400], F32R, name="wfT")
        nc.sync.dma_start(out=wfT[:], in_=wfTd.rearrange("(c p) o -> p c o", p=128))
        wtT = pc.tile([128, 200], F32, name="wtT")
        nc.sync.dma_start(out=wtT[:], in_=wtTd[:])
        wg1h = pc.tile([128, 512], F32R, name="wg1h")
        nc.sync.dma_start(out=wg1h[:], in_=wg1Td[0:128, :])
        wg1l = pc.tile([72, 512], F32R, name="wg1l")
        nc.sync.dma_start(out=wg1l[:], in_=wg1Td[128:200, :])
        wg2 = pc.tile([128, 4, 200], F32, name="wg2")
        nc.sync.dma_start(out=wg2[:], in_=wg2Td.rearrange("(c p) o -> p c o", p=128))
        wch, wcl = {}, {}
        for k in KSZ:
            wch[k] = pc.tile([128, k, 128], F32, name=f"wch{k}")
            nc.sync.dma_start(out=wch[k][:], in_=wcd[k].rearrange("k d m -> d k m")[0:128])
            wcl[k] = pc.tile([72, k, 128], F32, name=f"wcl{k}")
            nc.sync.dma_start(out=wcl[k][:], in_=wcd[k].rearrange("k d m -> d k m")[128:200])

        hbuf = pd.tile([NPC, PADF], F32, name="hbuf")
        hfull = pd.tile([NCORES * NPC, PADF], F32, name="hfull", addr_space="Shared")
        h2_all = pb.tile([128, NT * 200], F32, name="h2_all")
        ob_all = pb.tile([128, NT * B], F32, name="ob_all")

        do_gcn = parts in ("all", "gcn", "g1", "g1c")
        do_g1c = parts in ("all", "gcn", "g1c")
        do_g2 = parts in ("all", "gcn")
        do_cnn = parts in ("all", "cnn")
        # ---------------- CNN-a: embedding gather, xT, conv, xd
        xTh = pb.tile([128, B, 512], F32, name="xTh")
        xTl = pb.tile([72, B, 512], F32, name="xTl")
        for o in range(16 if do_cnn else 0):
            ei = pi.tile([128, 1], I32, name=f"ei{o}", tag="ei")
            nc.sync.dma_start(out=ei[:], in_=iseq[o * 128:(o + 1) * 128, :])
            eg = pw.tile([128, 200], F32, name=f"eg{o}", tag="eg")
            nc.gpsimd.indirect_dma_start(
                out=eg[:], out_offset=None, in_=emb[:],
                in_offset=bass.IndirectOffsetOnAxis(ap=ei[:, :1], axis=0))
            b, q = o // 4, o % 4
            tp = psB.tile([128, 128], F32, name=f"xtp{o}", tag="psS")
            nc.tensor.transpose(out=tp[:], in_=eg[:, 0:128], identity=ident[:])
            nc.scalar.copy(out=xTh[:, b, q * 128:(q + 1) * 128], in_=tp[:])
            tp2 = psB.tile([72, 128], F32, name=f"xtp2{o}", tag="psS")
            nc.tensor.transpose(out=tp2[:], in_=eg[:, 128:200], identity=ident[:])
            nc.scalar.copy(out=xTl[:, b, q * 128:(q + 1) * 128], in_=tp2[:])

        cT_all, xdh_all, xdl_all = {}, {}, {}

        def emit_conv(b):
            for ki, k in enumerate(KSZ):
                Lp = L - k + 1
                cps = psA.tile([128, 512], F32, name=f"cps{b}{k}", tag="psL")
                for i in range(k):
                    nc.tensor.matmul(out=cps[:, 0:Lp], lhsT=wch[k][:, i, :],
                                     rhs=xTh[:, b, i:i + Lp],
                                     start=(i == 0), stop=False)
                    nc.tensor.matmul(out=cps[:, 0:Lp], lhsT=wcl[k][:, i, :],
                                     rhs=xTl[:, b, i:i + Lp],
                                     start=False, stop=(i == k - 1))
                csb = pw.tile([128, 512], F32, name=f"csb{b}{k}", tag="csb")
                nc.scalar.activation(out=csb[:, 0:Lp], in_=cps[:, 0:Lp],
                                     func=AF.Relu, bias=cst[:, ki:ki + 1])
                # cT: [l, nk] chunks
                cT = pw.tile([128, 4, 128], F32R, name=f"cT{b}{k}", tag="cT", bufs=6)
                for lc in range(4):
                    l0 = lc * 128
                    ln = min(128, Lp - l0)
                    ctp = psB.tile([128, 128], F32, name=f"ctp{b}{k}{lc}", tag="psS")
                    nc.tensor.transpose(out=ctp[0:ln, :], in_=csb[:, l0:l0 + ln],
                                        identity=ident[:])
                    nc.vector.tensor_copy(out=cT[0:ln, lc, :], in_=ctp[0:ln, :])
                cT_all[b, k] = cT
                # xdT = tanh(Wt @ c + bt): [d, l]
                xh = psA.tile([128, 512], F32, name=f"xh{b}{k}", tag="psL")
                nc.tensor.matmul(out=xh[:, 0:Lp], lhsT=wtT[:, 0:128], rhs=csb[:, 0:Lp],
                                 start=True, stop=True)
                xdh = pw.tile([128, 512], F32R, name=f"xdh{b}{k}", tag="xdh", bufs=4)
                nc.scalar.activation(out=xdh[:, 0:Lp], in_=xh[:, 0:Lp], func=AF.Tanh,
                                     bias=cst[:, 3:4])
                xl = psA.tile([72, 512], F32, name=f"xl{b}{k}", tag="psL")
                nc.tensor.matmul(out=xl[:, 0:Lp], lhsT=wtT[:, 128:200], rhs=csb[:, 0:Lp],
                                 start=True, stop=True)
                xdl = pw.tile([72, 512], F32R, name=f"xdl{b}{k}", tag="xdl", bufs=4)
                nc.scalar.activation(out=xdl[:, 0:Lp], in_=xl[:, 0:Lp], func=AF.Tanh,
                                     bias=cst[0:72, 4:5])
                xdh_all[b, k] = xdh
                xdl_all[b, k] = xdl

        # ---------------- GCN gather machinery (shared for both phases)
        def gather_phase(Wt_, table, consume_tile):
            gtiles = [None] * len(ops_cols)
            emitted = [False] * len(ops_cols)
            woff = [0]
            offs = np.cumsum([0] + [c * 8 for c in ops_cols])

            def ensure(oi):
                if emitted[oi]:
                    return
                ncols = ops_cols[oi]
                it = pi.tile([128, JMAX * 8], I16, name=f"gi_{id(Wt_)}_{oi}", tag="gidx")
                nc.sync.dma_start(out=it[:, 0:ncols * 8],
                                  in_=Wt_[:, int(offs[oi]):int(offs[oi]) + ncols * 8])
                g = pg.tile([128, JMAX, PADF], F32, name=f"g_{id(Wt_)}_{oi}", tag="gbuf")
                nc.gpsimd.dma_gather(out_ap=g[:, 0:ncols, :], in_ap=table[:],
                                     idxs_ap=it[:, 0:ncols * 8],
                                     num_idxs=ncols * 128, num_idxs_reg=ncols * 128,
                                     elem_size=PADF)
                gtiles[oi] = g
                emitted[oi] = True

            for t in range(NT):
                for si, (oi, co, jn) in enumerate(segs[t]):
                    ensure(oi)
                consume_tile(t, [(gtiles[oi], co, jn) for (oi, co, jn) in segs[t]])

        def seg_reduce(dst_ap, pieces, tag):
            first = True
            for g, co, jn in pieces:
                srcv = g[:, co:co + jn, 0:200].rearrange("p j d -> p d j")
                if first:
                    nc.vector.reduce_sum(out=dst_ap, in_=srcv, axis=AX.X)
                    first = False
                else:
                    tmp = pw.tile([128, 200], F32, name=f"tmp{tag}", tag="redtmp")
                    nc.vector.reduce_sum(out=tmp[:], in_=srcv, axis=AX.X)
                    nc.vector.tensor_tensor(out=dst_ap, in0=dst_ap, in1=tmp[:],
                                            op=ALU.add)

        # ---------------- GCN phase 1
        aggTh = {}
        aggTl = {}

        def consume1(t, pieces):
            ng, tt = t // 4, t % 4
            if tt == 0:
                aggTh[ng] = pw.tile([128, 512], F32R, name=f"aggTh{ng}", tag="aggTh")
                aggTl[ng] = pw.tile([72, 512], F32R, name=f"aggTl{ng}", tag="aggTl")
            acc = pw.tile([128, 200], F32, name=f"acc1_{t}", tag="acc")
            seg_reduce(acc[:], pieces, f"a{t}")
            th = psG.tile([128, 128], F32, name=f"t1h{t}", tag="gS")
            nc.tensor.transpose(out=th[:], in_=acc[:, 0:128], identity=ident[:])
            nc.vector.tensor_copy(out=aggTh[ng][:, tt * 128:(tt + 1) * 128], in_=th[:])
            tl = psG.tile([72, 128], F32, name=f"t1l{t}", tag="gS")
            nc.tensor.transpose(out=tl[:], in_=acc[:, 128:200], identity=ident[:])
            nc.vector.tensor_copy(out=aggTl[ng][:, tt * 128:(tt + 1) * 128], in_=tl[:])
            if tt == 3:
                hsb = pw.tile([128, 4, 512], F32, name=f"hsb{ng}", tag="hsb", bufs=1)
                for hc in range(4):
                    hps = psG.tile([128, 512], F32, name=f"hps{ng}{hc}", tag="gL", bufs=1)
                    nc.tensor.matmul(out=hps[:], lhsT=wg1h[:, hc * 128:(hc + 1) * 128],
                                     rhs=aggTh[ng][:], start=True, stop=False)
                    nc.tensor.matmul(out=hps[:], lhsT=wg1l[:, hc * 128:(hc + 1) * 128],
                                     rhs=aggTl[ng][:], start=False, stop=True)
                    nc.scalar.activation(out=hsb[:, hc, :], in_=hps[:], func=AF.Relu,
                                         bias=cst[:, 5 + hc:6 + hc])
                for q in range(4):
                    tq = ng * 4 + q
                    hp = psG.tile([128, 200], F32, name=f"hp{tq}", tag="gS")
                    for hc in range(4):
                        nc.tensor.matmul(out=hp[:], lhsT=hsb[:, hc, q * 128:(q + 1) * 128],
                                         rhs=wg2[:, hc, :], start=(hc == 0), stop=(hc == 3))
                    ho = pw.tile([128, 200], F32, name=f"ho{tq}", tag="ho")
                    nc.scalar.copy(out=ho[:], in_=hp[:])
                    nc.sync.dma_start(out=hbuf[tq * 128:(tq + 1) * 128, 0:200], in_=ho[:])

        if do_gcn:
            gather_phase(W1, fpad, consume1)
            nc.sync.dma_start(out=hbuf[NREAL:NPC, :], in_=zeros_t[0:NPC - NREAL, :])
        if do_g1c:
            nc.gpsimd.collective_compute(
                "AllGather", ALU.bypass, replica_groups=[list(range(NCORES))],
                ins=[hbuf[:]], outs=[hfull[:]])
        if debug and do_gcn:
            nc.sync.dma_start(out=dbg["hb"][:], in_=hbuf[:])

        # ---------------- GCN phase 2 -> h2_all
        def consume2(t, pieces):
            sl = h2_all[:, t * 200:(t + 1) * 200]
            seg_reduce(sl, pieces, f"b{t}")
            nc.vector.tensor_tensor(out=sl, in0=sl, in1=cst[:, 9:209], op=ALU.add)

        if do_g2:
            gather_phase(W2, hfull, consume2)
        else:
            nc.vector.memset(h2_all[:], 0.0)
        if debug and do_g2:
            nc.sync.dma_start(out=dbg["h2"][:], in_=h2_all[:])

        # ---------------- CNN-b: scores/softmax/content/Wf/final dot
        for b in range(B if do_cnn else 0):
            emit_conv(b)
            for ng in range(NT // 4):
                n0 = ng * 512
                fTh = pw.tile([128, 512], F32R, name=f"fTh{b}{ng}", tag="fTh")
                nc.sync.dma_start(out=fTh[:], in_=featT[0:128, n0:n0 + 512])
                fTl = pw.tile([72, 512], F32R, name=f"fTl{b}{ng}", tag="fTl")
                nc.sync.dma_start(out=fTl[:], in_=featT[128:200, n0:n0 + 512])
                csc = pw.tile([128, 3, 512], F32R, name=f"csc{b}{ng}", tag="csc", bufs=1)
                for ki, k in enumerate(KSZ):
                    Lp = L - k + 1
                    xdh, xdl = xdh_all[b, k], xdl_all[b, k]
                    cT = cT_all[b, k]
                    expS = pw.tile([128, 4, 512], F32R, name=f"ex{b}{ng}{k}", tag="expS")
                    for lc in range(4):
                        l0 = lc * 128
                        ln = min(128, Lp - l0)
                        sps = psA.tile([128, 512], F32, name=f"sps{b}{ng}{k}{lc}", tag="psL")
                        nc.tensor.matmul(out=sps[0:ln, :], lhsT=xdh[:, l0:l0 + ln],
                                         rhs=fTh[:], start=True, stop=False)
                        nc.tensor.matmul(out=sps[0:ln, :], lhsT=xdl[:, l0:l0 + ln],
                                         rhs=fTl[:], start=False, stop=True)
                        nc.scalar.activation(out=expS[0:ln, lc, :], in_=sps[0:ln, :],
                                             func=AF.Exp)
                    sums = psA.tile([1, 512], F32, name=f"sm{b}{ng}{k}", tag="psL")
                    cop = psA.tile([128, 512], F32, name=f"cop{b}{ng}{k}", tag="psL")
                    for lc in range(4):
                        ln = min(128, Lp - lc * 128)
                        nc.tensor.matmul(out=sums[:], lhsT=ones_r[0:ln, :],
                                         rhs=expS[0:ln, lc, :],
                                         start=(lc == 0), stop=(lc == 3))
                        nc.tensor.matmul(out=cop[:], lhsT=cT[0:ln, lc, :],
                                         rhs=expS[0:ln, lc, :],
                                         start=(lc == 0), stop=(lc == 3))
                    rec = pw.tile([1, 512], F32R, name=f"rec{b}{ng}{k}", tag="rec")
                    with nc.allow_low_precision(reason="f32r storage is full fp32"):
                        nc.vector.reciprocal(out=rec[:], in_=sums[:])
                    rbc = psA.tile([128, 512], F32, name=f"rbc{b}{ng}{k}", tag="psL")
                    nc.tensor.matmul(out=rbc[:], lhsT=ones_row[:], rhs=rec[:],
                                     start=True, stop=True)
                    rbs = pw.tile([128, 512], F32, name=f"rbs{b}{ng}{k}", tag="rbs")
                    nc.scalar.copy(out=rbs[:], in_=rbc[:])
                    nc.vector.tensor_tensor(out=csc[:, ki, :], in0=cop[:], in1=rbs[:],
                                            op=ALU.mult)
                if debug and b == 0 and ng == 0:
                    nc.sync.dma_start(out=dbg["xc"][:],
                                      in_=csc[:].rearrange("p a b -> p (a b)").bitcast(F32))
                for q in range(4):
                    t = ng * 4 + q
                    xfp = psB.tile([128, 400], F32, name=f"xfp{b}{t}", tag="psS")
                    for ki in range(3):
                        nc.tensor.matmul(out=xfp[:], lhsT=csc[:, ki, q * 128:(q + 1) * 128],
                                         rhs=wfT[:, ki, :], start=(ki == 0), stop=False)
                    nc.tensor.matmul(out=xfp[:], lhsT=ones_row[:], rhs=bfr[:],
                                     start=False, stop=True)
                    xf = pw.tile([128, 400], F32, name=f"xf{b}{t}", tag="xf")
                    nc.scalar.activation(out=xf[:], in_=xfp[:], func=AF.Relu)
                    if debug and b == 0 and t == 0:
                        nc.sync.dma_start(out=dbg["xf"][:], in_=xf[:])
                    fst = pw.tile([128, 200], F32, name=f"fst{b}{t}", tag="fst")
                    nc.sync.dma_start(out=fst[:], in_=feat_s[t * 128:(t + 1) * 128, :])
                    junk = pw.tile([128, 400], F32, name=f"junk{b}{t}", tag="junk")
                    part = pw.tile([128, 2], F32, name=f"part{b}{t}", tag="part")
                    nc.vector.tensor_tensor(out=junk[:, 0:200], in0=xf[:, 0:200],
                                            in1=h2_all[:, t * 200:(t + 1) * 200],
                                            op=ALU.mult)
                    nc.vector.tensor_tensor(out=junk[:, 200:400], in0=xf[:, 200:400],
                                            in1=fst[:], op=ALU.mult)
                    nc.vector.reduce_sum(out=ob_all[:, t * B + b:t * B + b + 1],
                                         in_=junk[:], axis=AX.X)

        if not do_cnn:
            nc.vector.memset(ob_all[:], 0.0)
        for t in range(NT):
            osb = pw.tile([128, B], F32, name=f"osb{t}", tag="osb")
            nc.scalar.activation(out=osb[:], in_=ob_all[:, t * B:(t + 1) * B],
                                 func=AF.Sigmoid)
            nc.sync.dma_start(out=out[t * 128:(t + 1) * 128, :], in_=osb[:])


# ---------------------------------------------------------------- runner
def _run_spmd(nc, in_maps, timed_reps=0):
    import jax
    from jax.sharding import Mesh, PartitionSpec, NamedSharding
    from jax.experimental.shard_map import shard_map
    from concourse.bass2jax import install_neuronx_cc_hook, _bass_exec_p, partition_id_tensor

    install_neuronx_cc_hook()
    partition_name = nc.partition_id_tensor.name if nc.partition_id_tensor else None
    in_names, out_names, out_avals, zero_outs = [], [], [], []
    for alloc in nc.m.functions[0].allocations:
        if not isinstance(alloc, mybir.MemoryLocationSet):
            continue
        name = alloc.memorylocations[0].name
        if alloc.kind == "ExternalInput":
            if name != partition_name:
                in_names.append(name)
        elif alloc.kind == "ExternalOutput":
            out_names.append(name)
            out_avals.append(jax_shaped(alloc))
            zero_outs.append(np.zeros(tuple(alloc.tensor_shape),
                                      mybir.dt.np(alloc.dtype)))
    n_params, n_outs = len(in_names), len(out_avals)
    donate = tuple(range(n_params, n_params + n_outs))
    all_in = list(in_names) + list(out_names)
    if partition_name is not None:
        all_in.append(partition_name)

    def _body(*args):
        operands = list(args)
        if partition_name is not None:
            operands.append(partition_id_tensor())
        outs = _bass_exec_p.bind(
            *operands, out_avals=tuple(out_avals), in_names=tuple(all_in),
            out_names=tuple(out_names), lowering_input_output_aliases=(),
            sim_require_finite=False, sim_require_nnan=False, nc=nc)
        return tuple(outs)

    devices = jax.devices()[:NCORES]
    mesh = Mesh(np.asarray(devices), ("core",))
    specs = (PartitionSpec("core"),)
    sharded = jax.jit(
        shard_map(_body, mesh=mesh, in_specs=specs * (n_params + n_outs),
                  out_specs=specs * n_outs, check_rep=False),
        donate_argnums=donate, keep_unused=True)
    sh = NamedSharding(mesh, PartitionSpec("core"))
    per_core = [[np.asarray(m[nm]) for nm in in_names] for m in in_maps]
    dev_in = [jax.device_put(np.concatenate([per_core[c][i] for c in range(NCORES)],
                                            axis=0), sh)
              for i in range(len(in_names))]
    jax.block_until_ready(dev_in)

    def zeros():
        return [jax.device_put(
            np.zeros((NCORES * z.shape[0], *z.shape[1:]), z.dtype), sh)
            for z in zero_outs]

    times = []
    out_arrs = sharded(*dev_in, *zeros())
    jax.block_until_ready(out_arrs)
    if timed_reps:
        import time
        zs_all = [zeros() for _ in range(timed_reps)]
        jax.block_until_ready(zs_all)
        t0 = time.perf_counter()
        outs = [sharded(*dev_in, *zs) for zs in zs_all]
        jax.block_until_ready(outs)
        times.append((time.perf_counter() - t0) / timed_reps)
        out_arrs = outs[-1]
    res = [
        {nm: np.asarray(out_arrs[i]).reshape(NCORES, *out_avals[i].shape)[c]
         for i, nm in enumerate(out_names)}
        for c in range(NCORES)
    ]
    return res, times


def jax_shaped(alloc):
    import jax
    return jax.core.ShapedArray(tuple(alloc.tensor_shape), mybir.dt.np(alloc.dtype))


_CACHE = {}


def kernel(debug=False, timed_reps=0, parts="all", **inputs):
    in_maps, meta = _host_prep(inputs)
    key = tuple(meta["Ks"]) + (debug, parts)
    if key not in _CACHE:
        _CACHE[key] = build_program(meta, debug=debug, parts=parts)
    nc = _CACHE[key]
    res, times = _run_spmd(nc, in_maps, timed_reps=timed_reps)
    order = meta["order"]
    outf = np.zeros((B, N), np.float32)
    for c in range(NCORES):
        oc = res[c]["out"]                       # [NPC, B]
        nodes = order[c::NCORES]
        outf[:, nodes] = oc[:NREAL, :].T
    if debug or timed_reps:
        return outf, res, times
    return outf
